# revision 4
# baseline (speedup 1.0000x reference)
"""Trainium2 Bass kernel for nn_DerivativeEstimator (BDF2 ROM integrator).

Strategy: shard the hyper-reduction points (Ng) 8 ways across NeuronCores.
Each core holds 1/8 of W_all_u / W_all_v / P_proj resident in SBUF (bf16),
computes its partial RHS projection f^T = -(Pp_local @ concat_local) each
iteration, all-reduces the tiny (64, 32) partial across the 8 cores, and
every core redundantly runs the (cheap, fp32) BDF2 update so no state ever
leaves the chip.

Layouts keep the hyper-reduction points on SBUF/PSUM partitions end to end:
  z^T[pt, b]  = block-diag(W-tile) matmuls  (K=128 = 2x64, M=128 pts, N=32)
  products    = one bf16 tensor_tensor per (eq, pair-plane) group
  f^T[i, b]   = accumulated matmuls with lhsT = -P_proj^T tiles
  BDF2 update = fp32 DVE ops + one fp32 matmul with lhsT = [M^-T | M^-T]
so no transposes are ever needed.
"""

import sys

if "/opt/trn_rl_repo" not in sys.path:
    sys.path.insert(0, "/opt/trn_rl_repo")

import numpy as np
import ml_dtypes
from contextlib import ExitStack

import concourse.bass as bass
import concourse.bacc as bacc
import concourse.tile as tile
from concourse import mybir
from concourse.bass_utils import run_bass_kernel_spmd

P = 64          # reduced dim
B = 32          # batch of trajectories
ITERS = 49      # BDF2 steps after the implicit-Euler step
DT = 0.02
DT_SKIP = DT * 20
NG = 8000       # points per equation (u and v each)
NG_PAD = 8192
N_CORES = 8
NL = NG_PAD // N_CORES          # 1024 points per core per equation
N_PTILE = NL // 128             # 8 point-tiles of 128 per comp per eq
F32 = mybir.dt.float32
BF16 = mybir.dt.bfloat16

LAST_RESULT = None  # BassKernelResults of the most recent run (for test.py)
_CACHED_NC = None


def _bf16(x):
    return np.asarray(x, np.float32).astype(ml_dtypes.bfloat16)


def _build_bass():
    nc = bacc.Bacc("TRN2", target_bir_lowering=False, debug=False,
                   num_devices=N_CORES)
    # z-stage stationary weights: 64 block-diag lhsT tiles [128, 128]
    # (2 eqs x 4 comps x 8 point-tiles), partition-major in DRAM.
    wz = nc.declare_dram_parameter("wz", [128, 64, 128], BF16, isOutput=False)
    # projection lhsT tiles (-P_proj^T): 16 tiles [128, 64], partition-major.
    ppt = nc.declare_dram_parameter("ppt", [128, 16, 64], BF16, isOutput=False)
    a0t2 = nc.declare_dram_parameter("a0t2", [128, B], BF16, isOutput=False)
    a0t = nc.declare_dram_parameter("a0t", [P, B], F32, isOutput=False)
    m1t = nc.declare_dram_parameter("m1t", [P, 128], F32, isOutput=False)
    m2t = nc.declare_dram_parameter("m2t", [P, 128], F32, isOutput=False)
    dout = nc.declare_dram_parameter("dout", [P, B], F32, isOutput=True)
    rg = [list(range(N_CORES))]

    with tile.TileContext(nc) as tc, ExitStack() as ctx:
        consts = ctx.enter_context(tc.tile_pool(name="consts", bufs=1))
        zsbp = ctx.enter_context(tc.tile_pool(name="zsbp", bufs=2))
        prodp = ctx.enter_context(tc.tile_pool(name="prodp", bufs=2))
        state = ctx.enter_context(tc.tile_pool(name="state", bufs=3))
        small = ctx.enter_context(tc.tile_pool(name="small", bufs=3))
        zpsum = ctx.enter_context(
            tc.tile_pool(name="zpsum", bufs=4, space="PSUM"))
        fpsum = ctx.enter_context(
            tc.tile_pool(name="fpsum", bufs=2, space="PSUM"))
        apsum = ctx.enter_context(
            tc.tile_pool(name="apsum", bufs=2, space="PSUM"))
        dram = ctx.enter_context(tc.tile_pool(name="dram", bufs=1,
                                              space="DRAM"))

        wz_sb = consts.tile([128, 64, 128], BF16)
        nc.sync.dma_start(out=wz_sb[:, :, :], in_=wz[:, :, :])
        ppt_sb = consts.tile([128, 16, 64], BF16)
        nc.sync.dma_start(out=ppt_sb[:, :, :], in_=ppt[:, :, :])
        m1t_sb = consts.tile([P, 128], F32)
        nc.sync.dma_start(out=m1t_sb[:, :], in_=m1t[:, :])
        m2t_sb = consts.tile([P, 128], F32)
        nc.sync.dma_start(out=m2t_sb[:, :], in_=m2t[:, :])
        a0t_sb = consts.tile([P, B], F32)
        nc.sync.dma_start(out=a0t_sb[:, :], in_=a0t[:, :])
        at2_init = consts.tile([128, B], BF16)
        nc.sync.dma_start(out=at2_init[:, :], in_=a0t2[:, :])

        # python-side rolling state handles
        at2_bf = at2_init          # [128, B] bf16: [a^T; a^T]
        at_f32 = a0t_sb            # [P, B] fp32 current a^T
        at_prev = None             # [P, B] fp32 previous a^T
        f2dt_prev = None           # [P, B] fp32: 2*dt*f^T_prev

        for it in range(ITERS + 1):
            # ---- z-stage: 64 block-diag matmuls -> 4 PSUM banks ----
            zbank = []
            for eq in range(2):
                for half in range(2):     # half 0: comps(uu,vu); 1: (dx,dy)
                    zb = zpsum.tile([128, 512], F32, tag="zb")
                    for ci in range(2):
                        comp = half * 2 + ci
                        for j in range(N_PTILE):
                            t = eq * 32 + comp * 8 + j
                            nc.tensor.matmul(
                                out=zb[:, ci * 256 + j * 32:
                                       ci * 256 + j * 32 + 32],
                                lhsT=wz_sb[:, t, :],
                                rhs=at2_bf[:, :],
                                start=True, stop=True,
                            )
                    zbank.append(zb)

            # ---- evacuate to SBUF as bf16, then pair-products ----
            prods = []
            for eq in range(2):
                z0 = zsbp.tile([128, 512], BF16, tag="z0")
                nc.scalar.copy(out=z0[:, :], in_=zbank[2 * eq][:, :])
                z1 = zsbp.tile([128, 512], BF16, tag="z1")
                nc.scalar.copy(out=z1[:, :], in_=zbank[2 * eq + 1][:, :])
                pr = prodp.tile([128, 512], BF16, tag="pr")
                nc.vector.tensor_mul(pr[:, :], z0[:, :], z1[:, :])
                prods.append(pr)

            # ---- projection: accumulate 32 matmuls into f^T [64, 32] ----
            ft = fpsum.tile([P, B], F32, tag="ft")
            n_mm = 0
            for eq in range(2):
                for j in range(N_PTILE):
                    for plane in range(2):
                        nc.tensor.matmul(
                            out=ft[:, :],
                            lhsT=ppt_sb[:, eq * 8 + j, :],
                            rhs=prods[eq][:, plane * 256 + j * 32:
                                          plane * 256 + j * 32 + 32],
                            start=(n_mm == 0), stop=(n_mm == 31),
                        )
                        n_mm += 1

            # ---- all-reduce the partial f^T across the 8 cores ----
            cc_in = dram.tile([P, B], F32, name=f"ccin{it}")
            cc_out = dram.tile([P, B], F32, addr_space="Shared",
                               name=f"ccout{it}")
            ft_sb = small.tile([P, B], F32, tag="ftsb")
            nc.scalar.copy(out=ft_sb[:, :], in_=ft[:, :])
            nc.sync.dma_start(out=cc_in[:, :], in_=ft_sb[:, :])
            nc.gpsimd.collective_compute(
                "AllReduce", mybir.AluOpType.add, replica_groups=rg,
                ins=[cc_in[:, :]], outs=[cc_out[:, :]],
            )
            fsum = small.tile([P, B], F32, tag="fsum")
            nc.sync.dma_start(out=fsum[:, :], in_=cc_out[:, :])

            # ---- BDF2 (or Euler) update, fp32 ----
            r = small.tile([P, B], F32, tag="r")
            if it == 0:
                # r = a0 + dt*f
                nc.vector.scalar_tensor_tensor(
                    out=r[:, :], in0=fsum[:, :], scalar=float(DT),
                    in1=at_f32[:, :], op0=mybir.AluOpType.mult,
                    op1=mybir.AluOpType.add)
                mt = m1t_sb
            else:
                s1 = small.tile([P, B], F32, tag="s1")
                nc.vector.scalar_tensor_tensor(
                    out=s1[:, :], in0=at_f32[:, :], scalar=4.0,
                    in1=at_prev[:, :], op0=mybir.AluOpType.mult,
                    op1=mybir.AluOpType.subtract)
                s2 = small.tile([P, B], F32, tag="s2")
                nc.vector.scalar_tensor_tensor(
                    out=s2[:, :], in0=fsum[:, :], scalar=float(4.0 * DT),
                    in1=f2dt_prev[:, :], op0=mybir.AluOpType.mult,
                    op1=mybir.AluOpType.subtract)
                nc.vector.tensor_add(r[:, :], s1[:, :], s2[:, :])
                mt = m2t_sb
            f2dt = small.tile([P, B], F32, tag="f2dt")
            nc.vector.tensor_scalar_mul(f2dt[:, :], fsum[:, :],
                                        float(2.0 * DT))

            an = apsum.tile([128, B], F32, tag="an")
            nc.tensor.matmul(out=an[:, :], lhsT=mt[:, :], rhs=r[:, :],
                             start=True, stop=True)

            at2_next = state.tile([128, B], BF16, tag="at2")
            nc.scalar.copy(out=at2_next[:, :], in_=an[:, :])
            at_next = state.tile([P, B], F32, tag="atf")
            nc.scalar.copy(out=at_next[:, :], in_=an[:P, :])

            at_prev = at_f32
            at_f32 = at_next
            at2_bf = at2_next
            f2dt_prev = f2dt

        # ---- output: (a_final - a0) / DT_SKIP, transposed on host ----
        d = small.tile([P, B], F32, tag="d")
        nc.vector.tensor_sub(d[:, :], at_f32[:, :], a0t_sb[:, :])
        dscale = small.tile([P, B], F32, tag="dscale")
        nc.vector.tensor_scalar_mul(dscale[:, :], d[:, :],
                                    float(1.0 / DT_SKIP))
        nc.sync.dma_start(out=dout[:, :], in_=dscale[:, :])

    nc.finalize()
    return nc


def _prep_inputs(state, Ap, W_all_u, W_all_v, P_proj):
    a0 = np.asarray(state, np.float32)           # (B, P)
    Ap = np.asarray(Ap, np.float32)
    eye = np.eye(P, dtype=np.float32)
    m1 = (eye - np.float32(DT) * Ap).astype(np.float64)
    m2 = (np.float32(3.0) * eye - np.float32(2.0 * DT) * Ap).astype(np.float64)
    m1inv = np.linalg.inv(m1).astype(np.float32)
    m2inv = np.linalg.inv(m2).astype(np.float32)
    # lhsT for a_next^T = M^-1 @ r^T, output duplicated to 128 partitions
    m1t = np.concatenate([m1inv.T, m1inv.T], axis=1).copy()   # (64, 128)
    m2t = np.concatenate([m2inv.T, m2inv.T], axis=1).copy()

    a0t = np.ascontiguousarray(a0.T)                          # (64, 32)
    a0t2 = _bf16(np.concatenate([a0t, a0t], axis=0))          # (128, 32)

    def comp_pad(W):
        W = np.asarray(W, np.float32).reshape(4, NG, P)
        return np.pad(W, ((0, 0), (0, NG_PAD - NG), (0, 0)))

    Wu = comp_pad(W_all_u)      # (4, NG_PAD, 64)
    Wv = comp_pad(W_all_v)
    Pp = np.asarray(P_proj, np.float32)
    Ppu = np.pad(Pp[:, :NG], ((0, 0), (0, NG_PAD - NG)))      # (64, NG_PAD)
    Ppv = np.pad(Pp[:, NG:], ((0, 0), (0, NG_PAD - NG)))

    in_maps = []
    for c in range(N_CORES):
        sl = slice(c * NL, (c + 1) * NL)
        wz = np.zeros((128, 64, 128), np.float32)
        for eq, W in enumerate((Wu, Wv)):
            Wc = W[:, sl, :]                                  # (4, NL, 64)
            for comp in range(4):
                for j in range(N_PTILE):
                    blk = Wc[comp, j * 128:(j + 1) * 128, :]  # (128, 64)
                    t = eq * 32 + comp * 8 + j
                    wz[0:64, t, 0:64] = blk[0:64].T
                    wz[64:128, t, 64:128] = blk[64:128].T
        ppt = np.zeros((128, 16, 64), np.float32)
        for eq, Ppe in enumerate((Ppu, Ppv)):
            Pc = Ppe[:, sl]                                   # (64, NL)
            for j in range(N_PTILE):
                ppt[:, eq * 8 + j, :] = -Pc[:, j * 128:(j + 1) * 128].T
        in_maps.append({
            "wz": _bf16(wz),
            "ppt": _bf16(ppt),
            "a0t2": a0t2,
            "a0t": a0t,
            "m1t": m1t,
            "m2t": m2t,
        })
    return in_maps


def kernel(state, Ap, W_all_u, W_all_v, P_proj, _trace=False):
    global LAST_RESULT, _CACHED_NC
    if _CACHED_NC is None:
        _CACHED_NC = _build_bass()
    in_maps = _prep_inputs(state, Ap, W_all_u, W_all_v, P_proj)
    res = run_bass_kernel_spmd(_CACHED_NC, in_maps, list(range(N_CORES)),
                               trace=_trace)
    LAST_RESULT = res
    return np.ascontiguousarray(res.results[0]["dout"].T)


# revision 10
# speedup vs baseline: 2.1381x; 2.1381x over previous
"""Trainium2 Bass kernel for nn_DerivativeEstimator (BDF2 ROM integrator).

Strategy: shard the hyper-reduction points (Ng) 8 ways across NeuronCores.
Each core holds 1/8 of W_all_u / W_all_v / P_proj resident in SBUF (bf16),
computes its partial RHS projection f^T = -(Pp_local @ concat_local) each
iteration, all-reduces the tiny (64, 32) partial across the 8 cores, and
every core redundantly runs the (cheap, fp32) BDF2 update so no state ever
leaves the chip.

Layouts keep the hyper-reduction points on SBUF/PSUM partitions end to end:
  z^T[pt, b]  = block-diag(W-tile) matmuls  (K=128 = 2x64, M=128 pts, N=32)
  products    = one bf16 tensor_tensor per (eq, pair-plane) group
  f^T[i, b]   = accumulated matmuls with lhsT = -P_proj^T tiles
  BDF2 update = fp32 DVE ops + one fp32 matmul with lhsT = [M^-T | M^-T]
so no transposes are ever needed.
"""

import sys

if "/opt/trn_rl_repo" not in sys.path:
    sys.path.insert(0, "/opt/trn_rl_repo")

import numpy as np
import ml_dtypes
from contextlib import ExitStack

import concourse.bass as bass
import concourse.bacc as bacc
import concourse.tile as tile
from concourse import mybir
from concourse.bass_utils import run_bass_kernel_spmd

P = 64          # reduced dim
B = 32          # batch of trajectories
ITERS = 49      # BDF2 steps after the implicit-Euler step
DT = 0.02
DT_SKIP = DT * 20
NG = 8000       # points per equation (u and v each)
NG_PAD = 8192
N_CORES = 8
NL = NG_PAD // N_CORES          # 1024 points per core per equation
N_PTILE = NL // 128             # 8 point-tiles of 128 per comp per eq
F32 = mybir.dt.float32
BF16 = mybir.dt.bfloat16

LAST_RESULT = None  # BassKernelResults of the most recent run (for test.py)
_CACHED_NC = None


def _bf16(x):
    return np.asarray(x, np.float32).astype(ml_dtypes.bfloat16)


def _build_bass():
    nc = bacc.Bacc("TRN2", target_bir_lowering=False, debug=False,
                   num_devices=N_CORES)
    # z-stage stationary weights: 64 block-diag lhsT tiles [128, 128]
    # (2 eqs x 4 comps x 8 point-tiles), partition-major in DRAM.
    wz = nc.declare_dram_parameter("wz", [128, 64, 128], BF16, isOutput=False)
    # projection lhsT tiles (-P_proj^T): 16 tiles [128, 64], partition-major.
    ppt = nc.declare_dram_parameter("ppt", [128, 16, 64], BF16, isOutput=False)
    a0t2 = nc.declare_dram_parameter("a0t2", [128, B], BF16, isOutput=False)
    a0t = nc.declare_dram_parameter("a0t", [P, B], F32, isOutput=False)
    m1t = nc.declare_dram_parameter("m1t", [P, 128], F32, isOutput=False)
    m2t = nc.declare_dram_parameter("m2t", [P, 128], F32, isOutput=False)
    dout = nc.declare_dram_parameter("dout", [P, B], F32, isOutput=True)
    rg = [list(range(N_CORES))]

    with tile.TileContext(nc) as tc, ExitStack() as ctx:
        consts = ctx.enter_context(tc.tile_pool(name="consts", bufs=1))
        zsbp = ctx.enter_context(tc.tile_pool(name="zsbp", bufs=2))
        prodp = ctx.enter_context(tc.tile_pool(name="prodp", bufs=2))
        state = ctx.enter_context(tc.tile_pool(name="state", bufs=3))
        small = ctx.enter_context(tc.tile_pool(name="small", bufs=3))
        zpsum = ctx.enter_context(
            tc.tile_pool(name="zpsum", bufs=4, space="PSUM"))
        fpsum = ctx.enter_context(
            tc.tile_pool(name="fpsum", bufs=2, space="PSUM"))
        apsum = ctx.enter_context(
            tc.tile_pool(name="apsum", bufs=2, space="PSUM"))
        dram = ctx.enter_context(tc.tile_pool(name="dram", bufs=1,
                                              space="DRAM"))

        wz_sb = consts.tile([128, 64, 128], BF16)
        nc.sync.dma_start(out=wz_sb[:, :, :], in_=wz[:, :, :])
        ppt_sb = consts.tile([128, 16, 64], BF16)
        nc.sync.dma_start(out=ppt_sb[:, :, :], in_=ppt[:, :, :])
        m1t_sb = consts.tile([P, 128], F32)
        nc.sync.dma_start(out=m1t_sb[:, :], in_=m1t[:, :])
        m2t_sb = consts.tile([P, 128], F32)
        nc.sync.dma_start(out=m2t_sb[:, :], in_=m2t[:, :])
        a0t_sb = consts.tile([P, B], F32)
        nc.sync.dma_start(out=a0t_sb[:, :], in_=a0t[:, :])
        at2_init = consts.tile([128, B], BF16)
        nc.sync.dma_start(out=at2_init[:, :], in_=a0t2[:, :])

        # python-side rolling state handles
        at2_bf = at2_init          # [128, B] bf16: [a^T; a^T]
        at_f32 = a0t_sb            # [P, B] fp32 current a^T
        at_prev = None             # [P, B] fp32 previous a^T
        f2dt_prev = None           # [P, B] fp32: 2*dt*f^T_prev

        for it in range(ITERS + 1):
            # ---- z-stage: 64 block-diag matmuls -> 4 PSUM banks ----
            zbank = []
            for eq in range(2):
                for half in range(2):     # half 0: comps(uu,vu); 1: (dx,dy)
                    zb = zpsum.tile([128, 512], F32, tag="zb")
                    for ci in range(2):
                        comp = half * 2 + ci
                        for j in range(N_PTILE):
                            t = eq * 32 + comp * 8 + j
                            nc.tensor.matmul(
                                out=zb[:, ci * 256 + j * 32:
                                       ci * 256 + j * 32 + 32],
                                lhsT=wz_sb[:, t, :],
                                rhs=at2_bf[:, :],
                                start=True, stop=True,
                            )
                    zbank.append(zb)

            # ---- evacuate to SBUF as bf16, then pair-products ----
            prods = []
            for eq in range(2):
                z0 = zsbp.tile([128, 512], BF16, tag="z0")
                nc.scalar.copy(out=z0[:, :], in_=zbank[2 * eq][:, :])
                z1 = zsbp.tile([128, 512], BF16, tag="z1")
                nc.scalar.copy(out=z1[:, :], in_=zbank[2 * eq + 1][:, :])
                pr = prodp.tile([128, 512], BF16, tag="pr")
                nc.vector.tensor_mul(pr[:, :], z0[:, :], z1[:, :])
                prods.append(pr)

            # ---- projection: accumulate 16 matmuls into f^T [64, 32] ----
            # rhs covers both pair-planes at once via a strided [128, 2, 32]
            # AP; PSUM accumulation folds the two N-halves? No — matmul N=64
            # yields [64, 2, 32] outputs; use out AP [64, 2, 32] where both
            # halves land on the same [64, 32] region via step-0: illegal.
            # Instead keep two accumulating halves and let PSUM accumulate by
            # issuing the same out twice? Simplest correct: rhs [128, 64]
            # strided, out [64, 64] = two planes side by side, then one extra
            # reduction. Cheaper: keep 2 MMs but batch rhs slices
            # contiguously (plane-major already contiguous per plane).
            ft = fpsum.tile([P, 2, B], F32, tag="ft")
            n_mm = 0
            for eq in range(2):
                for j in range(N_PTILE):
                    pr3 = prods[eq].rearrange("p (pl x) -> p pl x", pl=2)
                    nc.tensor.matmul(
                        out=ft[:, :, :],
                        lhsT=ppt_sb[:, eq * 8 + j, :],
                        rhs=pr3[:, :, j * 32:j * 32 + 32],
                        start=(n_mm == 0), stop=(n_mm == 15),
                    )
                    n_mm += 1

            # ---- all-gather the partial f^T planes, reduce locally ----
            cc_in = dram.tile([P, 2 * B], F32, name=f"ccin{it}")
            cc_out = dram.tile([N_CORES, P, 2 * B], F32, addr_space="Shared",
                               name=f"ccout{it}")
            ft_sb = small.tile([P, 2 * B], F32, tag="ftsb")
            nc.scalar.copy(out=ft_sb[:, :], in_=ft[:, :, :])
            nc.sync.dma_start(out=cc_in[:, :], in_=ft_sb[:, :])
            nc.gpsimd.collective_compute(
                "AllGather", mybir.AluOpType.bypass, replica_groups=rg,
                ins=[cc_in[:, :]], outs=[cc_out[:, :, :]],
            )
            # load as [P, B, 16] (inner = rank x plane) and reduce
            fgath = small.tile([P, B, 2, N_CORES], F32, tag="fgath")
            for pl in range(2):
                src = bass.AP(
                    tensor=cc_out.tensor,
                    offset=cc_out.offset + pl * B,
                    ap=[[2 * B, P], [1, B], [P * 2 * B, N_CORES]],
                )
                nc.sync.dma_start(out=fgath[:, :, pl, :], in_=src)
            fsum = small.tile([P, B], F32, tag="fsum")
            nc.vector.tensor_reduce(
                out=fsum[:, :], in_=fgath.rearrange("p b pl r -> p b (pl r)"),
                axis=mybir.AxisListType.X, op=mybir.AluOpType.add)

            # ---- BDF2 (or Euler) update, fp32 ----
            r = small.tile([P, B], F32, tag="r")
            if it == 0:
                # r = a0 + dt*f
                nc.vector.scalar_tensor_tensor(
                    out=r[:, :], in0=fsum[:, :], scalar=float(DT),
                    in1=at_f32[:, :], op0=mybir.AluOpType.mult,
                    op1=mybir.AluOpType.add)
                mt = m1t_sb
            else:
                s1 = small.tile([P, B], F32, tag="s1")
                nc.vector.scalar_tensor_tensor(
                    out=s1[:, :], in0=at_f32[:, :], scalar=4.0,
                    in1=at_prev[:, :], op0=mybir.AluOpType.mult,
                    op1=mybir.AluOpType.subtract)
                s2 = small.tile([P, B], F32, tag="s2")
                nc.vector.scalar_tensor_tensor(
                    out=s2[:, :], in0=fsum[:, :], scalar=float(4.0 * DT),
                    in1=f2dt_prev[:, :], op0=mybir.AluOpType.mult,
                    op1=mybir.AluOpType.subtract)
                nc.vector.tensor_add(r[:, :], s1[:, :], s2[:, :])
                mt = m2t_sb
            f2dt = small.tile([P, B], F32, tag="f2dt")
            nc.vector.tensor_scalar_mul(f2dt[:, :], fsum[:, :],
                                        float(2.0 * DT))

            an = apsum.tile([128, B], F32, tag="an")
            nc.tensor.matmul(out=an[:, :], lhsT=mt[:, :], rhs=r[:, :],
                             start=True, stop=True)

            at2_next = state.tile([128, B], BF16, tag="at2")
            nc.scalar.copy(out=at2_next[:, :], in_=an[:, :])
            at_next = state.tile([P, B], F32, tag="atf")
            nc.scalar.copy(out=at_next[:, :], in_=an[:P, :])

            at_prev = at_f32
            at_f32 = at_next
            at2_bf = at2_next
            f2dt_prev = f2dt

        # ---- output: (a_final - a0) / DT_SKIP, transposed on host ----
        d = small.tile([P, B], F32, tag="d")
        nc.vector.tensor_sub(d[:, :], at_f32[:, :], a0t_sb[:, :])
        dscale = small.tile([P, B], F32, tag="dscale")
        nc.vector.tensor_scalar_mul(dscale[:, :], d[:, :],
                                    float(1.0 / DT_SKIP))
        nc.sync.dma_start(out=dout[:, :], in_=dscale[:, :])

    nc.finalize()
    return nc


def _prep_inputs(state, Ap, W_all_u, W_all_v, P_proj):
    a0 = np.asarray(state, np.float32)           # (B, P)
    Ap = np.asarray(Ap, np.float32)
    eye = np.eye(P, dtype=np.float32)
    m1 = (eye - np.float32(DT) * Ap).astype(np.float64)
    m2 = (np.float32(3.0) * eye - np.float32(2.0 * DT) * Ap).astype(np.float64)
    m1inv = np.linalg.inv(m1).astype(np.float32)
    m2inv = np.linalg.inv(m2).astype(np.float32)
    # lhsT for a_next^T = M^-1 @ r^T, output duplicated to 128 partitions
    m1t = np.concatenate([m1inv.T, m1inv.T], axis=1).copy()   # (64, 128)
    m2t = np.concatenate([m2inv.T, m2inv.T], axis=1).copy()

    a0t = np.ascontiguousarray(a0.T)                          # (64, 32)
    a0t2 = _bf16(np.concatenate([a0t, a0t], axis=0))          # (128, 32)

    def comp_pad(W):
        W = np.asarray(W, np.float32).reshape(4, NG, P)
        return np.pad(W, ((0, 0), (0, NG_PAD - NG), (0, 0)))

    Wu = comp_pad(W_all_u)      # (4, NG_PAD, 64)
    Wv = comp_pad(W_all_v)
    Pp = np.asarray(P_proj, np.float32)
    Ppu = np.pad(Pp[:, :NG], ((0, 0), (0, NG_PAD - NG)))      # (64, NG_PAD)
    Ppv = np.pad(Pp[:, NG:], ((0, 0), (0, NG_PAD - NG)))

    in_maps = []
    for c in range(N_CORES):
        sl = slice(c * NL, (c + 1) * NL)
        wz = np.zeros((128, 64, 128), np.float32)
        for eq, W in enumerate((Wu, Wv)):
            Wc = W[:, sl, :]                                  # (4, NL, 64)
            for comp in range(4):
                for j in range(N_PTILE):
                    blk = Wc[comp, j * 128:(j + 1) * 128, :]  # (128, 64)
                    t = eq * 32 + comp * 8 + j
                    wz[0:64, t, 0:64] = blk[0:64].T
                    wz[64:128, t, 64:128] = blk[64:128].T
        ppt = np.zeros((128, 16, 64), np.float32)
        for eq, Ppe in enumerate((Ppu, Ppv)):
            Pc = Ppe[:, sl]                                   # (64, NL)
            for j in range(N_PTILE):
                ppt[:, eq * 8 + j, :] = -Pc[:, j * 128:(j + 1) * 128].T
        in_maps.append({
            "wz": _bf16(wz),
            "ppt": _bf16(ppt),
            "a0t2": a0t2,
            "a0t": a0t,
            "m1t": m1t,
            "m2t": m2t,
        })
    return in_maps


def kernel(state, Ap, W_all_u, W_all_v, P_proj, _trace=False):
    global LAST_RESULT, _CACHED_NC
    if _CACHED_NC is None:
        _CACHED_NC = _build_bass()
    in_maps = _prep_inputs(state, Ap, W_all_u, W_all_v, P_proj)
    res = run_bass_kernel_spmd(_CACHED_NC, in_maps, list(range(N_CORES)),
                               trace=_trace)
    LAST_RESULT = res
    return np.ascontiguousarray(res.results[0]["dout"].T)
